# revision 2
# baseline (speedup 1.0000x reference)
"""MinkowskiConvolution forward on 8 TRN2 NeuronCores.

Computation (reference):
    out[n, o] = sum_k sum_c features[idx[k, n], c] * W[k, c, o]
with idx[k, n] == -1 meaning "no neighbor" (contributes zero).

Strategy:
  - Shard output points across the 8 cores (37504 padded points each);
    replicate the feature table (with an appended zero row) and the small
    kernel tensor. No collectives needed.
  - Host prep: remap idx -1 -> zero row, transpose idx to point-major,
    cast features/kernel to bf16, stack the 27 per-offset weight matrices
    (+1 zero pad) into 7 groups of 4 so each group's matmul contracts over
    4*32 = 128 channels.
  - Device, per 128-point tile:
      * 26 indirect DMAs (one per non-center offset) gather 128 rows each
        from the table in HBM: dest [128, 32] with one index per partition
        (the only indirect-DMA shape the TRN2 DGE unrolls correctly; it
        costs ~1.4us/instruction, which dominates the kernel).
      * the center offset is the identity map, so it is a dense DMA.
      * PE transposes the gathered [points, channels] blocks to
        [channels, points] via identity matmuls (bf16 PSUM), DVE copies
        them back to SBUF, and 7 stacked matmuls accumulate [128, 64] f32
        in PSUM; ACT copies out, HWDGE streams results to DRAM.
"""

import os
import sys
from contextlib import ExitStack

import numpy as np

sys.path.insert(0, os.path.dirname(os.path.abspath(__file__)))

import ml_dtypes

import concourse.bass as bass
import concourse.bacc as bacc
import concourse.mybir as mybir
import concourse.tile as tile
from concourse.bass_utils import run_bass_kernel_spmd
from concourse.masks import make_identity

P = 128
N = 300_000
K = 27
CENTER = K // 2
KPAD = 28          # 27 offsets + 1 zero-weight pad -> 7 groups of 4
NGROUPS = 7
INC = 32
OUTC = 64
NCORES = 8
NPAD = 300_032     # 8 * 37504
NP_CORE = NPAD // NCORES          # 37504
NTILES = NP_CORE // P             # 293
R = NPAD + 1                       # table rows + zero row (300033)
ZROW = NPAD

_BF16 = mybir.dt.bfloat16
_F32 = mybir.dt.float32
_I32 = mybir.dt.int32


def build_nc(ntiles=NTILES, r=R, core_row0=0, center_static=True):
    """Build + compile the per-core Bass program.

    core_row0: not needed — the center offset's rows are the shard's own
    rows; each core gets its own `row0` scalar via the idx input instead.
    To keep one program for all cores, the center rows are located via a
    dedicated `crow` input tensor holding the shard's global row offset
    baked into the DMA source by... simplest: the center DMA reads from a
    per-core `cfeat` DRAM input [ntiles*P, INC] (the shard's own feature
    rows, prepared on host).
    """
    nc = bacc.Bacc("TRN2", target_bir_lowering=False, debug=False)
    np_core = ntiles * P
    table = nc.dram_tensor("table", [r, INC], _BF16, kind="ExternalInput")
    idxT = nc.dram_tensor("idx", [np_core, K], _I32, kind="ExternalInput")
    cfeat = nc.dram_tensor("cfeat", [np_core, INC], _BF16, kind="ExternalInput")
    wst = nc.dram_tensor("wst", [P, NGROUPS * OUTC], _BF16, kind="ExternalInput")
    out = nc.dram_tensor("out", [np_core, OUTC], _F32, kind="ExternalOutput")

    with ExitStack() as ctx:
        tc = ctx.enter_context(tile.TileContext(nc))
        const = ctx.enter_context(tc.tile_pool(name="const", bufs=1))
        w_sb = const.tile([P, NGROUPS * OUTC], _BF16)
        nc.sync.dma_start(out=w_sb[:], in_=wst[:])
        ident = const.tile([P, P], _BF16)
        make_identity(nc, ident[:])

        idxp = ctx.enter_context(tc.tile_pool(name="idxp", bufs=4))
        gp = ctx.enter_context(tc.tile_pool(name="gp", bufs=4))
        gtp = ctx.enter_context(tc.tile_pool(name="gtp", bufs=3))
        osb = ctx.enter_context(tc.tile_pool(name="osb", bufs=4))
        pa = ctx.enter_context(tc.tile_pool(name="pa", bufs=2, space="PSUM"))
        pb = ctx.enter_context(tc.tile_pool(name="pb", bufs=2, space="PSUM"))
        po = ctx.enter_context(tc.tile_pool(name="po", bufs=2, space="PSUM"))

        for t in range(ntiles):
            idx_tile = idxp.tile([P, K], _I32, tag="idx")
            nc.sync.dma_start(out=idx_tile[:], in_=idxT[t * P:(t + 1) * P, :])
            g = gp.tile([P, KPAD * INC], _BF16, tag="g")
            for k in range(K):
                if center_static and k == CENTER:
                    nc.sync.dma_start(
                        out=g[:, k * INC:(k + 1) * INC],
                        in_=cfeat[t * P:(t + 1) * P, :],
                    )
                    continue
                nc.gpsimd.indirect_dma_start(
                    out=g[:, k * INC:(k + 1) * INC],
                    out_offset=None,
                    in_=table[:],
                    in_offset=bass.IndirectOffsetOnAxis(
                        ap=idx_tile[:, k:k + 1], axis=0
                    ),
                )
            # zero the 28th (pad) offset lane so group 6 contracts cleanly
            nc.vector.memset(g[:, K * INC:], 0.0)

            ps_a = pa.tile([P, 4 * P], _BF16, tag="pa")
            ps_b = pb.tile([P, 3 * P], _BF16, tag="pb")
            for gi in range(NGROUPS):
                dst = (
                    ps_a[:, gi * P:(gi + 1) * P]
                    if gi < 4
                    else ps_b[:, (gi - 4) * P:(gi - 3) * P]
                )
                nc.tensor.transpose(dst, g[:, gi * P:(gi + 1) * P], ident[:])
            gt = gtp.tile([P, KPAD * INC], _BF16, tag="gt")
            nc.vector.tensor_copy(out=gt[:, 0:4 * P], in_=ps_a[:])
            nc.vector.tensor_copy(out=gt[:, 4 * P:7 * P], in_=ps_b[:])
            ps_o = po.tile([P, OUTC], _F32, tag="po")
            for gi in range(NGROUPS):
                nc.tensor.matmul(
                    ps_o[:],
                    gt[:, gi * P:(gi + 1) * P],
                    w_sb[:, gi * OUTC:(gi + 1) * OUTC],
                    start=(gi == 0),
                    stop=(gi == NGROUPS - 1),
                )
            ot = osb.tile([P, OUTC], _F32, tag="ot")
            nc.scalar.copy(out=ot[:], in_=ps_o[:])
            nc.sync.dma_start(out=out[t * P:(t + 1) * P, :], in_=ot[:])
    nc.compile()
    return nc


def prep_inputs(features, kernel, neighbor_idx, npad=NPAD, r=R, zrow=ZROW):
    """Host-side prep: bf16 table with zero row, stacked weights, safe idx."""
    n = features.shape[0]
    table = np.zeros((r, INC), dtype=ml_dtypes.bfloat16)
    table[:n] = features.astype(ml_dtypes.bfloat16)

    wst = np.zeros((P, NGROUPS * OUTC), dtype=ml_dtypes.bfloat16)
    kb = kernel.astype(ml_dtypes.bfloat16)
    for k in range(K):
        g, a = divmod(k, 4)
        wst[a * INC:(a + 1) * INC, g * OUTC:(g + 1) * OUTC] = kb[k]

    idx_safe = np.full((K, npad), zrow, dtype=np.int32)
    idx_safe[:, :neighbor_idx.shape[1]] = np.where(
        neighbor_idx < 0, zrow, neighbor_idx
    )
    idx_t = np.ascontiguousarray(idx_safe.T)  # [npad, K] point-major
    return table, wst, idx_t


_nc_cache = {}


def prep_run(features, kernel, neighbor_idx):
    """Build (cached) program + per-core input maps for the given inputs."""
    center_static = bool(
        np.array_equal(
            neighbor_idx[CENTER], np.arange(neighbor_idx.shape[1], dtype=np.int32)
        )
    )
    key = ("full", center_static)
    if key not in _nc_cache:
        _nc_cache[key] = build_nc(center_static=center_static)
    nc = _nc_cache[key]

    table, wst, idx_t = prep_inputs(features, kernel, neighbor_idx)
    in_maps = []
    for ci in range(NCORES):
        lo = ci * NP_CORE
        in_maps.append(
            {
                "table": table,
                "wst": wst,
                "idx": idx_t[lo:lo + NP_CORE],
                "cfeat": np.ascontiguousarray(table[lo:lo + NP_CORE]),
            }
        )
    return nc, in_maps


def kernel(features, kernel, neighbor_idx):
    nc, in_maps = prep_run(features, kernel, neighbor_idx)
    res = run_bass_kernel_spmd(nc, in_maps, core_ids=list(range(NCORES)))
    out = np.concatenate([res.results[ci]["out"] for ci in range(NCORES)], axis=0)
    return np.ascontiguousarray(out[:N])


if __name__ == "__main__":
    rng = np.random.default_rng(1)
    f = rng.standard_normal((N, INC), dtype=np.float32)
    w = rng.standard_normal((K, INC, OUTC), dtype=np.float32) * 0.03
    idx = rng.integers(-1, N, size=(K, N)).astype(np.int32)
    idx[CENTER] = np.arange(N, dtype=np.int32)
    o = kernel(f, w, idx)
    print("out", o.shape, o.dtype, float(np.abs(o).mean()))



# revision 4
# speedup vs baseline: 1.0385x; 1.0385x over previous
"""MinkowskiConvolution forward, sparsity-compacted gather variant.

Baseline gathers 27 lanes x 293 tiles x 128 rows/core with ~50% of rows
being the zero row (invalid neighbors). SWDGE indirect-DMA instructions
(~1.41us each, 128 rows max) are the bottleneck, so instruction count is
everything. This variant packs each lane's VALID rows into full 128-row
gather tiles (~147/lane instead of 293), then expands the compacted rows
back into the per-point-tile lane layout with 0/1 expansion matmuls on
the PE (E[j,p] = (point[j] == p), generated on-chip by DVE is_equal from
a host-provided shifted-point column, exactly the tile_scatter_add
selection-matrix pattern). Expansion accumulates into a PSUM g-tile
[128 pts, 28*32] which then runs the baseline transpose + grouped-GEMM.

One program serves all 8 cores, so the ctile -> out-tile schedule is the
UNION of the 8 cores' spans; cores where a ctile doesn't touch an
out-tile contribute an all-zero E (data-masked, still correct).
"""

import os
import sys
from contextlib import ExitStack

import numpy as np

sys.path.insert(0, os.path.dirname(os.path.abspath(__file__)))

import ml_dtypes

import concourse.bass as bass
import concourse.bacc as bacc
import concourse.mybir as mybir
import concourse.tile as tile
from concourse.bass_utils import run_bass_kernel_spmd
from concourse.masks import make_identity

P = 128
N = 300_000
K = 27
CENTER = K // 2
KPAD = 28
NGROUPS = 7
INC = 32
OUTC = 64
NCORES = 8
NPAD = 300_032
NP_CORE = NPAD // NCORES          # 37504
NTILES = NP_CORE // P             # 293
R = NPAD + 1
ZROW = NPAD
PADPT = 1 << 20                   # sentinel point id for stream padding
ESPAN = 4                         # max out-tiles a ctile may span

_BF16 = mybir.dt.bfloat16
_F32 = mybir.dt.float32
_I32 = mybir.dt.int32

GLANES = [k for k in range(K) if k != CENTER]   # gathered lanes


def build_schedule(neighbor_idx):
    """Compact per-(core,lane) valid streams into ctiles with COMMON anchor
    windows: ctile i of lane k covers points [a_i, a_{i+1}) for ALL cores,
    windows sized so every core's valid count fits in 128. This keeps the
    ctile -> out-tile schedule identical across cores (one program).

    Returns (nctl, nct, cols_idx, cols_spt, sched, touch):
      sched[(k,i)] = (col, t0, t1)   out-tile span from the anchor window
      touch[T]     = [(k, i, col, toff, start, stop), ...]
    """
    idx_pad = np.full((K, NPAD), -1, dtype=np.int64)
    idx_pad[:, :N] = neighbor_idx
    pts_all = {}
    for ci in range(NCORES):
        lo = ci * NP_CORE
        for k in GLANES:
            col = idx_pad[k, lo:lo + NP_CORE]
            pts = np.nonzero(col >= 0)[0]
            pts_all[(ci, k)] = (pts, col[pts])

    # common anchor windows per lane
    anchors = {}
    for k in GLANES:
        ptrs = [0] * NCORES
        a = [0]
        while a[-1] < NP_CORE:
            # window may extend to just before any core's 129th valid point
            end = NP_CORE
            for ci in range(NCORES):
                pts = pts_all[(ci, k)][0]
                j = ptrs[ci] + P
                if j < len(pts):
                    end = min(end, int(pts[j]))
            assert end > a[-1]
            for ci in range(NCORES):
                pts = pts_all[(ci, k)][0]
                ptrs[ci] += int(np.searchsorted(pts[ptrs[ci]:], end))
            a.append(end)
        anchors[k] = a

    nctl = {k: len(anchors[k]) - 1 for k in GLANES}
    nct = sum(nctl.values())
    cols_idx = [np.full((nct, P), ZROW, dtype=np.int32) for _ in range(NCORES)]
    cols_spt = [np.full((nct, P), float(PADPT), dtype=np.float64)
                for _ in range(NCORES)]

    base = {}
    c = 0
    for k in GLANES:
        base[k] = c
        c += nctl[k]

    sched = {}
    for k in GLANES:
        a = anchors[k]
        for i in range(nctl[k]):
            col = base[k] + i
            t0 = a[i] // P
            t1 = (a[i + 1] - 1) // P
            assert t1 - t0 + 1 <= ESPAN, (k, i, t1 - t0 + 1)
            sched[(k, i)] = (col, t0, t1)
            for ci in range(NCORES):
                pts, rows = pts_all[(ci, k)]
                s0 = int(np.searchsorted(pts, a[i]))
                s1 = int(np.searchsorted(pts, a[i + 1]))
                m = s1 - s0
                assert m <= P, (k, i, ci, m)
                cols_idx[ci][col, :m] = rows[s0:s1]
                cols_spt[ci][col, :m] = pts[s0:s1] - t0 * P
    cols_idx = [np.ascontiguousarray(x.T) for x in cols_idx]
    cols_spt = [np.ascontiguousarray(x.T.astype(np.float32)) for x in cols_spt]

    # touch map: for each out-tile T, the expansion matmuls it needs
    per_tk = {}
    for (k, i), (col, t0, t1) in sched.items():
        for T in range(t0, t1 + 1):
            per_tk.setdefault((T, k), []).append((i, col, T - t0))
    touch = {T: [] for T in range(NTILES)}
    for k in GLANES:
        for T in range(NTILES):
            lst = sorted(per_tk.get((T, k), []))
            assert lst, (T, k)
            for j, (i, col, toff) in enumerate(lst):
                touch[T].append(
                    (k, i, col, toff, j == 0, j == len(lst) - 1)
                )
    return nctl, nct, cols_idx, cols_spt, sched, touch


def build_nc2(nct, nctl, sched, touch):
    nc = bacc.Bacc("TRN2", target_bir_lowering=False, debug=False)
    table = nc.dram_tensor("table", [R, INC], _BF16, kind="ExternalInput")
    idxc = nc.dram_tensor("idxc", [P, nct], _I32, kind="ExternalInput")
    sptc = nc.dram_tensor("sptc", [P, nct], _F32, kind="ExternalInput")
    iotad = nc.dram_tensor("iotad", [P, ESPAN * P], _F32, kind="ExternalInput")
    cfeat = nc.dram_tensor("cfeat", [NP_CORE, INC], _BF16, kind="ExternalInput")
    wst = nc.dram_tensor("wst", [P, NGROUPS * OUTC], _BF16, kind="ExternalInput")
    out = nc.dram_tensor("out", [NP_CORE, OUTC], _F32, kind="ExternalOutput")

    with ExitStack() as ctx:
        tc = ctx.enter_context(tile.TileContext(nc))
        const = ctx.enter_context(tc.tile_pool(name="const", bufs=1))
        w_sb = const.tile([P, NGROUPS * OUTC], _BF16)
        nc.sync.dma_start(out=w_sb[:], in_=wst[:])
        ident = const.tile([P, P], _BF16)
        make_identity(nc, ident[:])
        idx_all = const.tile([P, nct], _I32)
        nc.sync.dma_start(out=idx_all[:], in_=idxc[:, :])
        spt_all = const.tile([P, nct], _F32)
        nc.sync.dma_start(out=spt_all[:], in_=sptc[:, :])
        iota = const.tile([P, ESPAN * P], _F32)
        nc.sync.dma_start(out=iota[:], in_=iotad[:, :])

        gp = ctx.enter_context(tc.tile_pool(name="gp", bufs=4))
        ep = ctx.enter_context(tc.tile_pool(name="ep", bufs=3))
        gtp = ctx.enter_context(tc.tile_pool(name="gtp", bufs=3))
        osb = ctx.enter_context(tc.tile_pool(name="osb", bufs=4))
        pg = ctx.enter_context(tc.tile_pool(name="pg", bufs=2, space="PSUM"))
        pa = ctx.enter_context(tc.tile_pool(name="pa", bufs=2, space="PSUM"))
        po = ctx.enter_context(tc.tile_pool(name="po", bufs=2, space="PSUM"))

        g_tiles = {}
        e_tiles = {}
        next_ct = {k: 0 for k in GLANES}

        def ensure_gathered(k, upto_i):
            while next_ct[k] <= upto_i:
                i = next_ct[k]
                col, t0u, t1u = sched[(k, i)]
                span = t1u - t0u + 1
                g = gp.tile([P, INC], _BF16, tag=f"g{k}")
                nc.gpsimd.indirect_dma_start(
                    out=g[:],
                    out_offset=None,
                    in_=table[:],
                    in_offset=bass.IndirectOffsetOnAxis(
                        ap=idx_all[:, col:col + 1], axis=0
                    ),
                )
                e = ep.tile([P, ESPAN * P], _BF16, tag=f"e{k}")
                nc.vector.tensor_tensor(
                    out=e[:, :span * P],
                    in0=spt_all[:, col:col + 1].to_broadcast([P, span * P]),
                    in1=iota[:, :span * P],
                    op=mybir.AluOpType.is_equal,
                )
                g_tiles[(k, i)] = g
                e_tiles[(k, i)] = e
                next_ct[k] = i + 1

        for T in range(NTILES):
            ps_g = pg.tile([P, KPAD * INC], _F32, tag="pg")
            for (k, i, col, toff, st, sp) in touch[T]:
                ensure_gathered(k, i)
                nc.tensor.matmul(
                    ps_g[:, k * INC:(k + 1) * INC],
                    e_tiles[(k, i)][:, toff * P:(toff + 1) * P],
                    g_tiles[(k, i)][:],
                    start=st,
                    stop=sp,
                )
            gt = gtp.tile([P, KPAD * INC], _BF16, tag="gt")
            nc.vector.tensor_copy(out=gt[:], in_=ps_g[:])
            # center lane: dense identity rows, overwrite slice CENTER
            nc.sync.dma_start(
                out=gt[:, CENTER * INC:(CENTER + 1) * INC],
                in_=cfeat[T * P:(T + 1) * P, :],
            )
            nc.vector.memset(gt[:, K * INC:], 0.0)

            ps_a = pa.tile([P, KPAD * INC], _BF16, tag="pa")
            for gi in range(NGROUPS):
                nc.tensor.transpose(
                    ps_a[:, gi * P:(gi + 1) * P],
                    gt[:, gi * P:(gi + 1) * P],
                    ident[:],
                )
            gtt = gtp.tile([P, KPAD * INC], _BF16, tag="gtt")
            nc.vector.tensor_copy(out=gtt[:], in_=ps_a[:])
            ps_o = po.tile([P, OUTC], _F32, tag="po")
            for gi in range(NGROUPS):
                nc.tensor.matmul(
                    ps_o[:],
                    gtt[:, gi * P:(gi + 1) * P],
                    w_sb[:, gi * OUTC:(gi + 1) * OUTC],
                    start=(gi == 0),
                    stop=(gi == NGROUPS - 1),
                )
            ot = osb.tile([P, OUTC], _F32, tag="ot")
            nc.scalar.copy(out=ot[:], in_=ps_o[:])
            nc.sync.dma_start(out=out[T * P:(T + 1) * P, :], in_=ot[:])
    nc.compile()
    return nc


def prep_tables(features, kernel):
    table = np.zeros((R, INC), dtype=ml_dtypes.bfloat16)
    table[:features.shape[0]] = features.astype(ml_dtypes.bfloat16)
    wst = np.zeros((P, NGROUPS * OUTC), dtype=ml_dtypes.bfloat16)
    kb = kernel.astype(ml_dtypes.bfloat16)
    for k in range(K):
        g, a = divmod(k, 4)
        wst[a * INC:(a + 1) * INC, g * OUTC:(g + 1) * OUTC] = kb[k]
    return table, wst


_cache = {}


def prep_run(features, kernel, neighbor_idx):
    import hashlib

    key = hashlib.md5(np.ascontiguousarray(neighbor_idx)).hexdigest()
    if _cache.get("key") != key:
        _cache.clear()
        _cache["key"] = key
        _cache["sched"] = build_schedule(neighbor_idx)
    nctl, nct, cols_idx, cols_spt, sched, touch = _cache["sched"]
    if "nc" not in _cache:
        _cache["nc"] = build_nc2(nct, nctl, sched, touch)
    nc = _cache["nc"]

    table, wst = prep_tables(features, kernel)
    iota = np.broadcast_to(
        np.arange(ESPAN * P, dtype=np.float32), (P, ESPAN * P)
    ).copy()
    in_maps = []
    for ci in range(NCORES):
        lo = ci * NP_CORE
        in_maps.append({
            "table": table,
            "wst": wst,
            "idxc": cols_idx[ci],
            "sptc": cols_spt[ci],
            "iotad": iota,
            "cfeat": np.ascontiguousarray(table[lo:lo + NP_CORE]),
        })
    return nc, in_maps


def kernel(features, kernel, neighbor_idx):
    nc, in_maps = prep_run(features, kernel, neighbor_idx)
    res = run_bass_kernel_spmd(nc, in_maps, core_ids=list(range(NCORES)))
    out = np.concatenate([res.results[ci]["out"] for ci in range(NCORES)], axis=0)
    return np.ascontiguousarray(out[:N])


if __name__ == "__main__":
    rng = np.random.default_rng(1)
    f = rng.standard_normal((N, INC), dtype=np.float32)
    w = rng.standard_normal((K, INC, OUTC), dtype=np.float32) * 0.03
    idx = rng.integers(-1, N, size=(K, N)).astype(np.int32)
    idx[CENTER] = np.arange(N, dtype=np.int32)
    o = kernel(f, w, idx)
    print("out", o.shape, o.dtype, float(np.abs(o).mean()))
